# revision 14
# baseline (speedup 1.0000x reference)
"""Trainium2 Bass kernel for nn_ConstraintAwareBiasing.

Computes bias[b, n, i, j] = temp[n] * (relu(relu(hi[b,i] + hj[b,j]) @ W2 + b2) @ W3 + b3)[n]
with hi = x @ W1[:128] + b1, hj = x @ W1[128:], masked by `mask`.

Strategy (8 NeuronCores):
  - Shard the (b, i) query axis: core = b*4 + chunk, each core owns 128 i-rows
    against all 512 j for one batch element.
  - Host precomputes hi/hj (tiny [512,128] matmuls), folds head_temperatures
    into W3, adds b3*temp and applies the mask on the host. hi is shipped as
    bf16 so the s1 tensor_scalar runs in the DVE 4x packed mode.
  - On device, per pair of query rows (i=2p, 2p+1):
      h1 = relu(hjT + hi_col)        DVE/GPSIMD tensor_scalar (bf16 4x on DVE)
      q  = W2^T @ h1                 PE matmul -> PSUM pair tile [128, 2S]
      h2 = relu(q + b2)              ACT/DVE PSUM->SBUF pass per pair
      p2[32c:32c+16] = W3'^T @ h2    PE pair matmul (1024-col bf16 moving
                                     operand); 4 pairs (8 i) share one
                                     2-bank PSUM tile via tile_position
      s5: slab[.., g8] = p2 + b3'    one [128, 2S] evacuation per 8 rows
      per 4 g8-groups: 4 DMAs (one per col block c) ship the slab from
      sync/gpsimd queues; dram-side layout is (n, c, g8, e, j) so each
      DMA is 16 partitions x 16KB-contiguous runs.
    W3 of pair p-1 is emitted interleaved with stage 1 of pair p
    (software pipelining). A burst of dummy matmuls at the start runs
    during the input-DMA wait to release the PE HAM clock gate early.
"""

import numpy as np
import ml_dtypes

import concourse.bass as bass
import concourse.tile as tile
import concourse.mybir as mybir
from concourse import bacc
from concourse.bass_utils import run_bass_kernel_spmd

BF16 = ml_dtypes.bfloat16

B, S, D = 2, 512, 128          # batch, seq, state dim
H, NH = 128, 16                # hidden, heads
N_CORES = 8
CHUNKS = N_CORES // B          # i-chunks per batch element
I_PER_CORE = S // CHUNKS       # 128
PAIRS = I_PER_CORE // 2        # 64 pairs of i-rows
G8 = I_PER_CORE // 8           # 16 groups of 8 i-rows (one 2-bank PSUM tile)
NEG_INF = float("-inf")

_CACHE: dict = {}

# Engine-assignment patterns.
# s1 per i: "v" = VectorE, "g" = GpSimdE (7.5us/instr -- unusable), "a" = ScalarE
S1_PAT = ["v"] * 8
# s3 per pair index: "a" = ScalarE, "v" = VectorE  (13a : 3v)
S3_PAT = ["a", "a", "a", "v", "a", "a", "a", "a",
          "v", "a", "a", "a", "a", "v", "a", "a"]
# s5 per g8 group (8a : 8v)
S5_PAT = ["a", "v"]
# number of warm-up matmuls (HAM clock-gate release) during input DMA wait
WARM_MM = 12


def _build_bass():
    nc = bacc.Bacc("TRN2")
    dt = mybir.dt
    hj_d = nc.dram_tensor("hj", (H, S), dt.bfloat16, kind="ExternalInput")
    hi_d = nc.dram_tensor("hi", (H, I_PER_CORE), dt.float32, kind="ExternalInput")
    w2_d = nc.dram_tensor("w2", (H, H), dt.bfloat16, kind="ExternalInput")
    w3_d = nc.dram_tensor("w3", (H, NH), dt.bfloat16, kind="ExternalInput")
    b2_d = nc.dram_tensor("b2", (H, 1), dt.float32, kind="ExternalInput")
    # device output layout: (n, c, g8, e, j) with i = 8*g8 + 2*c + e
    out_d = nc.dram_tensor("out", (NH, 4, G8, 2, S), dt.float32,
                           kind="ExternalOutput")

    relu = mybir.ActivationFunctionType.Relu
    add, amax = mybir.AluOpType.add, mybir.AluOpType.max

    with tile.TileContext(nc) as tc:
        with tc.tile_pool(name="singles", bufs=1) as singles, \
             tc.tile_pool(name="h1p", bufs=10) as h1p, \
             tc.tile_pool(name="h2p", bufs=6) as h2p, \
             tc.tile_pool(name="otp", bufs=3) as otp, \
             tc.tile_pool(name="ps1", bufs=2, space="PSUM") as ps1, \
             tc.tile_pool(name="ps2", bufs=2, space="PSUM") as ps2:
            hj = singles.tile([H, S], dt.bfloat16)
            hi = singles.tile([H, I_PER_CORE], dt.float32)
            w2 = singles.tile([H, H], dt.bfloat16)
            w3 = singles.tile([H, NH], dt.bfloat16)
            b2 = singles.tile([H, 1], dt.float32)
            warm = singles.tile([128, 128], dt.bfloat16)
            # dummy relu first: pulls the ~1.3us ACT table load into the
            # input-DMA wait window instead of serializing at the first s3
            warmf = singles.tile([128, 1], dt.float32)
            nc.vector.memset(warmf[:], 0.0)
            nc.scalar.activation(out=warmf[:], in_=warmf[:], func=relu)
            # warm-up matmuls: the PE HAM clock gate needs ~3.4us of
            # sustained activity to release (1.2 -> 2.4 GHz); burn it while
            # the input DMAs are in flight.
            nc.gpsimd.memset(warm[:], 0.0)
            wq = ps1.tile([H, 2 * S], dt.float32, name="wq", tag="q")
            for _w in range(WARM_MM):
                nc.tensor.matmul(wq[:, 0:128], lhsT=warm[:], rhs=warm[:],
                                 start=True, stop=True)
            nc.sync.dma_start(out=hj[:], in_=hj_d[:])
            nc.sync.dma_start(out=hi[:], in_=hi_d[:])
            for t, d in [(w2, w2_d), (w3, w3_d), (b2, b2_d)]:
                nc.sync.dma_start(out=t[:], in_=d[:])

            # software pipeline at pair granularity: the W3 matmul of pair
            # p-1 is emitted between stage 1 (s1 + W2) and s3 of pair p so
            # the in-order PE stream never head-of-line blocks.
            pend = None        # (pair_idx, h2_tile) awaiting W3
            p2_state = {"tile": None}

            def w3_mm(pi, h2t):
                c = pi % 4
                if c == 0:
                    p2_state["tile"] = ps2.tile([128, 2 * S], dt.float32,
                                                name="p2", tag="p2")
                p2 = p2_state["tile"]
                for e in range(2):
                    nc.tensor.matmul(p2[32 * c:32 * c + NH, e * S:(e + 1) * S],
                                     lhsT=w3[:], rhs=h2t[:, e * S:(e + 1) * S],
                                     start=True, stop=True,
                                     tile_position=(0, 32 * c))
                return p2

            def s5_and_dma(pi, p2):
                # pair pi just finished the g8 group g = pi // 4: evacuate the
                # 2-bank PSUM tile to SBUF (DMA cannot read PSUM), then 4 DMAs
                # -- one per 32-row col block -- ship it; per partition n the
                # dst is a contiguous (e, j) run of 4KB.
                if pi % 4 != 3:
                    return
                g = pi // 4
                o = otp.tile([128, 2 * S], dt.float32, name="o", tag="o")
                if S5_PAT[g % len(S5_PAT)] == "v":
                    nc.vector.tensor_copy(out=o[:], in_=p2[:])
                else:
                    nc.scalar.copy(out=o[:], in_=p2[:])
                for c in range(4):
                    dst = out_d[:, c, g]
                    src = o[32 * c:32 * c + NH, :]
                    eng = nc.sync if c % 2 == 0 else nc.gpsimd
                    eng.dma_start(out=dst, in_=src)

            for p in range(PAIRS + 1):
                if p < PAIRS:
                    q = ps1.tile([H, 2 * S], dt.float32, name="q", tag="q")
                    h2 = h2p.tile([H, 2 * S], dt.bfloat16, name="h2", tag="h2")
                    for e in range(2):
                        i = 2 * p + e
                        h1 = h1p.tile([H, S], dt.bfloat16)
                        s1_eng = {"v": nc.vector, "g": nc.gpsimd,
                                  "a": nc.scalar}[S1_PAT[i % len(S1_PAT)]]
                        if s1_eng is nc.scalar:
                            nc.scalar.activation(out=h1[:], in_=hj[:], func=relu,
                                                 bias=hi[:, i:i + 1], scale=1.0)
                        else:
                            s1_eng.tensor_scalar(
                                out=h1[:], in0=hj[:], scalar1=hi[:, i:i + 1],
                                scalar2=0.0, op0=add, op1=amax)
                        nc.tensor.matmul(
                            q[:, e * S:(e + 1) * S],
                            lhsT=w2[:], rhs=h1[:], start=True, stop=True)

                if pend is not None:
                    p2 = w3_mm(*pend)

                if p < PAIRS:
                    if S3_PAT[p % len(S3_PAT)] == "a":
                        nc.scalar.activation(out=h2[:], in_=q[:],
                                             func=relu, bias=b2[:], scale=1.0)
                    else:
                        nc.vector.tensor_scalar(
                            out=h2[:], in0=q[:], scalar1=b2[:, 0:1],
                            scalar2=0.0, op0=add, op1=amax)

                if pend is not None:
                    s5_and_dma(pend[0], p2)
                pend = (p, h2) if p < PAIRS else None
    nc.compile()
    return nc


def _host_prep(inputs):
    x = np.asarray(inputs["state_embeddings"], dtype=np.float32)   # [B, S, D]
    W1 = np.asarray(inputs["W1"], dtype=np.float32)                # [2D, H]
    b1 = np.asarray(inputs["b1"], dtype=np.float32)                # [H]
    W2 = np.asarray(inputs["W2"], dtype=np.float32)                # [H, H]
    b2 = np.asarray(inputs["b2"], dtype=np.float32)                # [H]
    W3 = np.asarray(inputs["W3"], dtype=np.float32)                # [H, NH]
    b3 = np.asarray(inputs["b3"], dtype=np.float32)                # [NH]
    temp = np.asarray(inputs["head_temperatures"], dtype=np.float32)  # [NH]

    hi = x @ W1[:D] + b1                                           # [B, S, H]
    hj = x @ W1[D:]                                                # [B, S, H]
    w3p = (W3 * temp[None, :]).astype(BF16)                        # temp folded in
    b3p = b3 * temp                                                # added on host

    b2col = np.ascontiguousarray(b2.reshape(H, 1))

    in_maps = []
    for core in range(N_CORES):
        b, chunk = divmod(core, CHUNKS)
        i0 = chunk * I_PER_CORE
        in_maps.append({
            "hj": np.ascontiguousarray(hj[b].T).astype(BF16),               # [H, S]
            "hi": np.ascontiguousarray(hi[b, i0:i0 + I_PER_CORE].T,
                                       dtype=np.float32),
            "w2": W2.astype(BF16),
            "w3": w3p,
            "b2": b2col,
        })
    return in_maps, b3p


def _assemble(results, inputs, b3p):
    mask = np.asarray(inputs["mask"])
    out = np.empty((B, NH, S, S), dtype=np.float32)
    for core in range(N_CORES):
        b, chunk = divmod(core, CHUNKS)
        i0 = chunk * I_PER_CORE
        # core result: [NH, 4, G8, 2, S] with i = 8*g8 + 2*c + e
        dev = results[core]["out"]
        out[b, :, i0:i0 + I_PER_CORE, :] = dev.transpose(0, 2, 1, 3, 4).reshape(
            NH, I_PER_CORE, S)
    if b3p.any():
        out += b3p[None, :, None, None]
    if not mask.all():
        out = np.where(mask[:, None, :, :], out, np.float32(NEG_INF))
    return out


def _get_nc():
    if "nc" not in _CACHE:
        _CACHE["nc"] = _build_bass()
    return _CACHE["nc"]


def run(inputs, trace=False):
    nc = _get_nc()
    in_maps, b3p = _host_prep(inputs)
    res = run_bass_kernel_spmd(nc, in_maps, core_ids=list(range(N_CORES)),
                               trace=trace)
    out = _assemble(res.results, inputs, b3p)
    return out, res


def kernel(**inputs) -> np.ndarray:
    out, _ = run(inputs, trace=False)
    return out
